# revision 1
# baseline (speedup 1.0000x reference)
"""DRGRU (diffusion-conv GRU cell) Trainium2 kernel.

Per-core (8 cores, one batch sample each):
  A0 = diag(1/colsum(adj+I)) @ (adj+I),  A1 = diag(1/colsum(adj^T+I)) @ (adj^T+I)
  gconv(x) = [x, A0 x, A0 x, A0^2 x] @ W0^T + [x, A1 x, A1 x, A1^2 x] @ W1^T + b
  value = sigmoid(gconv1(cat(xi, xh)));  r,u = split(value)
  c = tanh(gconv2(cat(xi, r*xh)));  out = u*xh + (1-u)*c

Device layout strategy: diffusion hops compute z^T = act.T @ adjI_or[j,i]
(stationary = activation node-tile, moving = un-normalized adjacency), with the
row-normalization 1/d applied as a per-output-column scale folded into the
PSUM->SBUF copy (d is computed on device via ones-vector matmuls + reciprocal +
outer-product broadcast).  Projections contract features with small stationary
weights against feat-major activations.  PE transposes flip layouts between
hops.  All matmul operands are float32r (TF32-like, full PE rate at >=256 free).
"""

import os

import numpy as np
import ml_dtypes

import concourse.bacc as bacc
import concourse.mybir as mybir
from concourse import tile
from concourse.bass_utils import run_bass_kernel_spmd

B, N, D = 8, 1024, 64
F = 2 * D       # 128 per-node features into gconv1
NT = N // 128   # 8 node tiles
O1, O2 = 2 * D, D

F32 = mybir.dt.float32

_DT_NAME = os.environ.get("DRGRU_DT", "f32r")
if _DT_NAME == "bf16":
    DT, NPDT, FREE = mybir.dt.bfloat16, ml_dtypes.bfloat16, 512
else:
    DT, NPDT, FREE = mybir.dt.float32r, np.float32, 512
CH = [(i, min(i + FREE, N)) for i in range(0, N, FREE)]
# route layout transposes through the DMA xbar (bf16 only) instead of the PE
TRD = os.environ.get("DRGRU_TR", "pe") == "dma"

_cache: dict = {}


def _build_nc():
    nc = bacc.Bacc("TRN2", target_bir_lowering=False, debug=False, num_devices=8)

    a0t_d = nc.declare_dram_parameter("a0t", [N, N], DT, isOutput=False)
    a1t_d = nc.declare_dram_parameter("a1t", [N, N], DT, isOutput=False)
    xi_d = nc.declare_dram_parameter("xi", [N, D], F32, isOutput=False)
    xh_d = nc.declare_dram_parameter("xh", [N, D], F32, isOutput=False)
    xit_d = nc.declare_dram_parameter("xit", [D, N], F32, isOutput=False)
    xht_d = nc.declare_dram_parameter("xht", [D, N], F32, isOutput=False)
    w0t_d = nc.declare_dram_parameter("w0t", [4 * F, O1], F32, isOutput=False)
    w1t_d = nc.declare_dram_parameter("w1t", [4 * F, O1], F32, isOutput=False)
    wc0t_d = nc.declare_dram_parameter("wc0t", [4 * F, O2], F32, isOutput=False)
    wc1t_d = nc.declare_dram_parameter("wc1t", [4 * F, O2], F32, isOutput=False)
    b0_d = nc.declare_dram_parameter("b0", [O1, 1], F32, isOutput=False)
    b1_d = nc.declare_dram_parameter("b1", [O1, 1], F32, isOutput=False)
    bc0_d = nc.declare_dram_parameter("bc0", [O2, 1], F32, isOutput=False)
    bc1_d = nc.declare_dram_parameter("bc1", [O2, 1], F32, isOutput=False)
    id_d = nc.declare_dram_parameter("ident", [128, 128], F32, isOutput=False)
    out_d = nc.declare_dram_parameter("out", [N, D], F32, isOutput=True)

    with tile.TileContext(nc) as tc:
        with (
            tc.tile_pool(name="sb", bufs=1) as sb,
            tc.tile_pool(name="zp", bufs=4) as zp,
            tc.tile_pool(name="nmp", bufs=2) as nmp,
            tc.tile_pool(name="ph", bufs=2, space="PSUM") as ph,
            tc.tile_pool(name="pp", bufs=1, space="PSUM") as pp,
            tc.tile_pool(name="pd", bufs=2, space="PSUM") as pd,
        ):
            # ---------------- input DMAs ----------------
            a0sb = sb.tile([128, NT, N], DT, tag="a0sb")
            a1sb = sb.tile([128, NT, N], DT, tag="a1sb")
            for t in range(NT):
                nc.sync.dma_start(a0sb[:, t, :], a0t_d[t * 128 : (t + 1) * 128, :])
                nc.sync.dma_start(a1sb[:, t, :], a1t_d[t * 128 : (t + 1) * 128, :])

            w0sb = sb.tile([128, 4, O1], F32, tag="w0sb")
            w1sb = sb.tile([128, 4, O1], F32, tag="w1sb")
            wc0sb = sb.tile([128, 4, O2], F32, tag="wc0sb")
            wc1sb = sb.tile([128, 4, O2], F32, tag="wc1sb")
            nc.sync.dma_start(w0sb[:], w0t_d[:].rearrange("(f m) o -> f m o", m=4))
            nc.sync.dma_start(w1sb[:], w1t_d[:].rearrange("(f m) o -> f m o", m=4))
            nc.sync.dma_start(wc0sb[:], wc0t_d[:].rearrange("(f m) o -> f m o", m=4))
            nc.sync.dma_start(wc1sb[:], wc1t_d[:].rearrange("(f m) o -> f m o", m=4))

            b0sb = sb.tile([O1, 1], F32, tag="b0sb")
            b1sb = sb.tile([O1, 1], F32, tag="b1sb")
            bc0sb = sb.tile([O2, 1], F32, tag="bc0sb")
            bc1sb = sb.tile([O2, 1], F32, tag="bc1sb")
            nc.sync.dma_start(b0sb[:], b0_d[:])
            nc.sync.dma_start(b1sb[:], b1_d[:])
            nc.sync.dma_start(bc0sb[:], bc0_d[:])
            nc.sync.dma_start(bc1sb[:], bc1_d[:])

            xcst = sb.tile([128, NT, F], F32, tag="xcst")
            nc.sync.dma_start(
                xcst[:, :, 0:D], xi_d[:].rearrange("(t p) d -> p t d", p=128)
            )
            nc.sync.dma_start(
                xcst[:, :, D:F], xh_d[:].rearrange("(t p) d -> p t d", p=128)
            )
            xcTst = sb.tile([128, N], F32, tag="xcTst")
            nc.sync.dma_start(xcTst[0:D, :], xit_d[:])
            nc.sync.dma_start(xcTst[D:F, :], xht_d[:])
            xhT0 = sb.tile([D, N], F32, tag="xhT0")  # xh^T at base partition 0
            nc.sync.dma_start(xhT0[:], xht_d[:])
            identf = sb.tile([128, 128], F32, tag="identf")
            nc.sync.dma_start(identf[:], id_d[:])

            # ---------------- small prep (DVE) ----------------
            ident = sb.tile([128, 128], DT, tag="ident")
            nc.vector.tensor_copy(ident[:], identf[:])
            xc = sb.tile([128, NT, F], DT, tag="xc")
            nc.vector.tensor_copy(xc[:], xcst[:])
            xcT = sb.tile([128, N], DT, tag="xcT")
            nc.vector.tensor_copy(xcT[:], xcTst[:])

            wx1 = sb.tile([128, O1], DT, tag="wx1")
            wz10 = sb.tile([128, O1], DT, tag="wz10")
            wz20 = sb.tile([128, O1], DT, tag="wz20")
            wz11 = sb.tile([128, O1], DT, tag="wz11")
            wz21 = sb.tile([128, O1], DT, tag="wz21")
            nc.vector.tensor_add(wx1[:], w0sb[:, 0, :], w1sb[:, 0, :])
            nc.vector.tensor_add(wz10[:], w0sb[:, 1, :], w0sb[:, 2, :])
            nc.vector.tensor_copy(wz20[:], w0sb[:, 3, :])
            nc.vector.tensor_add(wz11[:], w1sb[:, 1, :], w1sb[:, 2, :])
            nc.vector.tensor_copy(wz21[:], w1sb[:, 3, :])
            vx1 = sb.tile([128, O2], DT, tag="vx1")
            vz10 = sb.tile([128, O2], DT, tag="vz10")
            vz20 = sb.tile([128, O2], DT, tag="vz20")
            vz11 = sb.tile([128, O2], DT, tag="vz11")
            vz21 = sb.tile([128, O2], DT, tag="vz21")
            nc.vector.tensor_add(vx1[:], wc0sb[:, 0, :], wc1sb[:, 0, :])
            nc.vector.tensor_add(vz10[:], wc0sb[:, 1, :], wc0sb[:, 2, :])
            nc.vector.tensor_copy(vz20[:], wc0sb[:, 3, :])
            nc.vector.tensor_add(vz11[:], wc1sb[:, 1, :], wc1sb[:, 2, :])
            nc.vector.tensor_copy(vz21[:], wc1sb[:, 3, :])
            bias1 = sb.tile([O1, 1], F32, tag="bias1")
            bias2 = sb.tile([O2, 1], F32, tag="bias2")
            nc.vector.tensor_add(bias1[:], b0sb[:], b1sb[:])
            nc.vector.tensor_add(bias2[:], bc0sb[:], bc1sb[:])

            # ------- phase 1: stream A slices; d-sums + hop1 + direct proj ---
            # d1[i] = sum_j a0t[j,i] = rowsum(adj+I)[i]  -> scales chain 1
            # d0[i] = sum_j a1t[j,i] = colsum(adj+I)[i]  -> scales chain 0
            pre1 = pp.tile([O1, N], F32, tag="pre")
            for c0, c1 in CH:  # direct term opens the pre1 accumulation group
                nc.tensor.matmul(pre1[:, c0:c1], wx1[:], xcT[:, c0:c1],
                                 start=True, stop=False)
            xc2T = sb.tile([128, N], DT, tag="xc2T")
            nc.vector.tensor_copy(xc2T[0:D, :], xcT[0:D, :])  # xi^T half, early

            # degree sums as DVE row-reductions, streamed with the DMA:
            #   rowsum(a0t)[j] = colsum(adj+I)[j] = d0[j]   (chain-0 scale)
            #   rowsum(a1t)[j] = rowsum(adj+I)[j] = d1[j]   (chain-1 scale)
            dnm = sb.tile([128, 2 * NT], F32, tag="dnm")  # [:,0:8]=d0, [:,8:16]=d1
            ph1a = ph.tile([128, N], F32, tag="hop")
            ph1b = ph.tile([128, N], F32, tag="hop")
            AX = mybir.AxisListType.X
            for t in range(NT):
                st, sp = (t == 0), (t == NT - 1)
                nc.vector.reduce_sum(dnm[:, t : t + 1], a0sb[:, t, :], axis=AX)
                for c0, c1 in CH:
                    nc.tensor.matmul(ph1a[:, c0:c1], xc[:, t, :], a0sb[:, t, c0:c1],
                                     start=st, stop=sp)
                nc.vector.reduce_sum(dnm[:, NT + t : NT + t + 1], a1sb[:, t, :], axis=AX)
                for c0, c1 in CH:
                    nc.tensor.matmul(ph1b[:, c0:c1], xc[:, t, :], a1sb[:, t, c0:c1],
                                     start=st, stop=sp)

            dinv_nm = sb.tile([128, 2 * NT], F32, tag="dinv_nm")
            nc.vector.reciprocal(dinv_nm[:], dnm[:])
            # flip 1/d to free-major via one small PE transpose, then
            # partition-broadcast each 128-node block via DMA
            ptr_d = pd.tile([128, 128], F32, tag="pd")
            nc.tensor.transpose(
                ptr_d[0 : 2 * NT, :], dinv_nm[:], identf[:]
            )
            dinv_fm = sb.tile([2 * NT, 128], F32, tag="dinv_fm")
            nc.vector.tensor_copy(dinv_fm[:], ptr_d[0 : 2 * NT, :])
            d0inv_1N = sb.tile([1, N], F32, tag="d0inv_1N")
            d1inv_1N = sb.tile([1, N], F32, tag="d1inv_1N")
            for t in range(NT):
                nc.sync.dma_start(
                    d0inv_1N[0:1, t * 128 : (t + 1) * 128], dinv_fm[t : t + 1, :]
                )
                nc.sync.dma_start(
                    d1inv_1N[0:1, t * 128 : (t + 1) * 128],
                    dinv_fm[NT + t : NT + t + 1, :],
                )
            bc0 = sb.tile([128, N], F32, tag="bc0")   # bc0[p,i] = 1/d0[i]
            bc1 = sb.tile([128, N], F32, tag="bc1")   # bc1[p,i] = 1/d1[i]
            nc.gpsimd.partition_broadcast(bc0[:], d0inv_1N[0:1, :])
            nc.gpsimd.partition_broadcast(bc1[:], d1inv_1N[0:1, :])

            def zscale(ps, scale):
                """PSUM hop result * per-column 1/d  -> SBUF DT (chunked)."""
                z = zp.tile([128, N], DT, tag="z")
                for c0, c1 in CH:
                    nc.vector.tensor_mul(z[:, c0:c1], ps[:, c0:c1], scale[:, c0:c1])
                return z

            def proj(pre, w, rhs, stop=False):
                for c0, c1 in CH:
                    nc.tensor.matmul(pre[:, c0:c1], w[:], rhs[:, c0:c1],
                                     start=False, stop=stop)

            def tr_pipeline(srcT, dt, o_parts=128, consume=None):
                """srcT [o_parts, N] -> node-major [128, NT, o_parts] (dtype dt).

                consume(t, nm) is invoked right after tile t lands so PE work
                (e.g. the next hop's matmuls over slice t) pipelines with the
                per-tile transposes.
                """
                nm = nmp.tile([128, NT, o_parts], dt, tag=f"nm{o_parts}")
                idap = identf if dt == F32 else ident
                for t in range(NT):
                    if TRD and dt == mybir.dt.bfloat16:
                        nc.sync.dma_start(
                            nm[:, t, :], srcT[:, t * 128 : (t + 1) * 128],
                            transpose=True,
                        )
                    else:
                        ptr = pd.tile([128, 128], dt, tag="pd")
                        nc.tensor.transpose(
                            ptr[0:128, 0:o_parts],
                            srcT[:, t * 128 : (t + 1) * 128],
                            idap[0:o_parts, 0:o_parts],
                        )
                        nc.vector.tensor_copy(nm[:, t, :], ptr[0:128, 0:o_parts])
                    if consume is not None:
                        consume(t, nm)
                return nm

            def hop2_consumer(ph2, asb):
                def consume(t, nm):
                    st, sp = (t == 0), (t == NT - 1)
                    for c0, c1 in CH:
                        nc.tensor.matmul(ph2[:, c0:c1], nm[:, t, :],
                                         asb[:, t, c0:c1], start=st, stop=sp)
                return consume

            # ---------------- gconv 1 (continued) ----------------
            z1c0 = zscale(ph1a, bc0)
            z1c1 = zscale(ph1b, bc1)
            proj(pre1, wz10, z1c0)
            proj(pre1, wz11, z1c1)
            ph2a = ph.tile([128, N], F32, tag="hop")
            ph2b = ph.tile([128, N], F32, tag="hop")
            tr_pipeline(z1c0, DT, consume=hop2_consumer(ph2a, a0sb))
            tr_pipeline(z1c1, DT, consume=hop2_consumer(ph2b, a1sb))
            z2c0 = zscale(ph2a, bc0)
            z2c1 = zscale(ph2b, bc1)
            proj(pre1, wz20, z2c0)
            proj(pre1, wz21, z2c1, stop=True)

            # split sigmoid into r/u halves so downstream tensor-tensor ops see
            # matching base partitions (HW: both SB inputs must share base)
            Sig = mybir.ActivationFunctionType.Sigmoid
            val_r = sb.tile([D, N], F32, tag="val_r")
            val_u = sb.tile([D, N], F32, tag="val_u")
            nc.scalar.activation(val_r[:], pre1[0:D, :], Sig, bias=bias1[0:D, :])
            nc.scalar.activation(val_u[:], pre1[D:F, :], Sig, bias=bias1[D:F, :])

            # xc2^T = [xi^T ; (r*xh)^T]   (xi^T half copied earlier)
            nc.vector.tensor_mul(xc2T[D:F, :], val_r[:], xhT0[:])

            # ---------------- gconv 2 ----------------
            pre2 = pp.tile([O2, N], F32, tag="pre")
            for c0, c1 in CH:
                nc.tensor.matmul(pre2[:, c0:c1], vx1[:], xc2T[:, c0:c1],
                                 start=True, stop=False)
            ph1a2 = ph.tile([128, N], F32, tag="hop")
            ph1b2 = ph.tile([128, N], F32, tag="hop")

            def hop1_both(t, nm):
                st, sp = (t == 0), (t == NT - 1)
                for c0, c1 in CH:
                    nc.tensor.matmul(ph1a2[:, c0:c1], nm[:, t, :],
                                     a0sb[:, t, c0:c1], start=st, stop=sp)
                for c0, c1 in CH:
                    nc.tensor.matmul(ph1b2[:, c0:c1], nm[:, t, :],
                                     a1sb[:, t, c0:c1], start=st, stop=sp)

            tr_pipeline(xc2T, DT, consume=hop1_both)
            z1c0_2 = zscale(ph1a2, bc0)
            z1c1_2 = zscale(ph1b2, bc1)
            proj(pre2, vz10, z1c0_2)
            proj(pre2, vz11, z1c1_2)
            ph2a2 = ph.tile([128, N], F32, tag="hop")
            ph2b2 = ph.tile([128, N], F32, tag="hop")
            tr_pipeline(z1c0_2, DT, consume=hop2_consumer(ph2a2, a0sb))
            tr_pipeline(z1c1_2, DT, consume=hop2_consumer(ph2b2, a1sb))
            z2c0_2 = zscale(ph2a2, bc0)
            z2c1_2 = zscale(ph2b2, bc1)
            proj(pre2, vz20, z2c0_2)
            proj(pre2, vz21, z2c1_2, stop=True)

            cT = sb.tile([O2, N], F32, tag="cT")
            nc.scalar.activation(
                cT[:], pre2[:], mybir.ActivationFunctionType.Tanh, bias=bias2[:]
            )

            # ---------------- combine: out = u*xh + (1-u)*c ----------------
            outT = sb.tile([O2, N], F32, tag="outT")
            nc.vector.tensor_sub(outT[:], xhT0[:], cT[:])             # xh - c
            nc.vector.tensor_mul(outT[:], val_u[:], outT[:])          # u*(xh-c)
            nc.vector.tensor_add(outT[:], outT[:], cT[:])             # + c
            out_nm = tr_pipeline(outT, F32, o_parts=O2)
            nc.sync.dma_start(
                out_d[:].rearrange("(t p) d -> p t d", p=128), out_nm[:]
            )

    nc.finalize()
    return nc


def _prep_inputs(inputs, hx, adj, W0, b0, W1, b1, Wc0, bc0, Wc1, bc1):
    """Host-side layout prep -> per-core input maps (no math beyond adj + I)."""
    eye = np.eye(N, dtype=np.float32)
    ident = np.eye(128, dtype=np.float32)
    shared = {
        "w0t": np.ascontiguousarray(W0.T).astype(np.float32),
        "w1t": np.ascontiguousarray(W1.T).astype(np.float32),
        "wc0t": np.ascontiguousarray(Wc0.T).astype(np.float32),
        "wc1t": np.ascontiguousarray(Wc1.T).astype(np.float32),
        "b0": b0.reshape(O1, 1).astype(np.float32),
        "b1": b1.reshape(O1, 1).astype(np.float32),
        "bc0": bc0.reshape(O2, 1).astype(np.float32),
        "bc1": bc1.reshape(O2, 1).astype(np.float32),
        "ident": ident,
    }
    in_maps = []
    xi_all = inputs.reshape(B, N, D)
    xh_all = hx.reshape(B, N, D)
    for b in range(B):
        adjI = adj[b] + eye
        m = dict(shared)
        m["a1t"] = np.ascontiguousarray(adjI).astype(NPDT)
        m["a0t"] = np.ascontiguousarray(adjI.T).astype(NPDT)
        m["xi"] = np.ascontiguousarray(xi_all[b]).astype(np.float32)
        m["xh"] = np.ascontiguousarray(xh_all[b]).astype(np.float32)
        m["xit"] = np.ascontiguousarray(xi_all[b].T).astype(np.float32)
        m["xht"] = np.ascontiguousarray(xh_all[b].T).astype(np.float32)
        in_maps.append(m)
    return in_maps


def kernel(**inputs) -> np.ndarray:
    args = {k: np.asarray(v) for k, v in inputs.items()}
    if "nc" not in _cache:
        _cache["nc"] = _build_nc()
    nc = _cache["nc"]
    in_maps = _prep_inputs(
        args["inputs"], args["hx"], args["adj"],
        args["W0"], args["b0"], args["W1"], args["b1"],
        args["Wc0"], args["bc0"], args["Wc1"], args["bc1"],
    )
    res = run_bass_kernel_spmd(nc, in_maps, list(range(B)))
    out = np.stack([res.results[b]["out"].reshape(N * D) for b in range(B)])
    return out.astype(np.float32)



# revision 27
# speedup vs baseline: 1.9817x; 1.9817x over previous
"""DRGRU (diffusion-conv GRU cell) Trainium2 kernel.

Per-core (8 cores, one batch sample each):
  A0 = diag(1/colsum(adj+I)) @ (adj+I),  A1 = diag(1/colsum(adj^T+I)) @ (adj^T+I)
  gconv(x) = [x, A0 x, A0 x, A0^2 x] @ W0^T + [x, A1 x, A1 x, A1^2 x] @ W1^T + b
  value = sigmoid(gconv1(cat(xi, xh)));  r,u = split(value)
  c = tanh(gconv2(cat(xi, r*xh)));  out = u*xh + (1-u)*c

Layout: diffusion hops compute z^T = act_nodemajor.T @ adjT_unnormalized with
the row-normalization 1/d applied as a per-output-column scale on the
PSUM->SBUF copy (zscale).  Projections contract features with small stationary
weights against feat-major activations.  PE transposes flip z^T back to
node-major for the next hop's stationary.  All matmul operands bf16.

Schedule: one packed descriptor carries xc/xcT/ident, then a0 streams in
pairs and a1 in quads on the SP queue (weights/biases/xh^T trail -- they are
needed late).  While a0 streams: hop1-chain0 (PE), row sums for d0 (Act,
activation accum), and a pairwise add tree for d1 = colsum (DVE).  bc0 comes
from a tiny DVE 32x32-block transpose + per-tile gpsimd broadcasts; bc1 from
one gpsimd partition_all_reduce + a [128,N] reciprocal, so both scale tiles
are ready before hop1-chain0's zscale is needed.  While a1 streams, chain0's
zscale -> transposes -> hop2 interleave with hop1-chain1.  Transposes per z
run as one block (pd bufs=4) with Pool evacuating; gconv2's xi-half
transposes run in the idle front phase.  Output is feature-major [D, N],
computed in two column chunks (combine on DVE and Pool in parallel), and the
host transposes.
"""

import numpy as np
import ml_dtypes

import concourse.bacc as bacc
import concourse.mybir as mybir
from concourse import bass_isa, tile
from concourse.bass_utils import run_bass_kernel_spmd

B, N, D = 8, 1024, 64
F = 2 * D       # 128 per-node features into gconv1
NT = N // 128   # 8 node tiles
O1, O2 = 2 * D, D

F32 = mybir.dt.float32
BF16 = mybir.dt.bfloat16
NPBF = ml_dtypes.bfloat16
FREE = 512
CH = [(i, min(i + FREE, N)) for i in range(0, N, FREE)]
PK0 = 2 * N + 128          # xc | xcT | identb
WPK = 2 * (4 * O1) + 2 * (4 * O2)  # w0 | w1 | wc0 | wc1 (feat-major blocks)

_cache: dict = {}


def _build_nc():
    nc = bacc.Bacc("TRN2", target_bir_lowering=False, debug=False, num_devices=8)

    pk0_d = nc.declare_dram_parameter("pk0", [128, PK0], BF16, isOutput=False)
    a0t_d = nc.declare_dram_parameter("a0t", [N, N], BF16, isOutput=False)
    a1t_d = nc.declare_dram_parameter("a1t", [N, N], BF16, isOutput=False)
    wpk_d = nc.declare_dram_parameter("wpk", [128, WPK], BF16, isOutput=False)
    fpk_d = nc.declare_dram_parameter("fpk", [128, 4], F32, isOutput=False)
    xht_d = nc.declare_dram_parameter("xht", [D, N], F32, isOutput=False)
    out_d = nc.declare_dram_parameter("out", [D, N], F32, isOutput=True)

    with tile.TileContext(nc) as tc:
        with (
            tc.tile_pool(name="sb", bufs=1) as sb,
            tc.tile_pool(name="zp", bufs=4) as zp,
            tc.tile_pool(name="nmp", bufs=3) as nmp,
            tc.tile_pool(name="ph", bufs=2, space="PSUM") as ph,
            tc.tile_pool(name="pp", bufs=1, space="PSUM") as pp,
            tc.tile_pool(name="pd", bufs=2, space="PSUM") as pd,
        ):
            # ---------------- input DMAs (single SP queue, in need order) ---
            pk0 = sb.tile([128, PK0], BF16, tag="pk0")
            a0sb = sb.tile([128, NT, N], BF16, tag="a0sb")
            a1sb = sb.tile([128, NT, N], BF16, tag="a1sb")

            def a0_pair(p):
                nc.sync.dma_start(
                    a0sb[:, 2 * p : 2 * p + 2, :],
                    a0t_d[256 * p : 256 * (p + 1), :].rearrange(
                        "(t p) n -> p t n", p=128),
                )

            nc.sync.dma_start(pk0[:], pk0_d[:])
            xc = pk0[:, 0:N]                    # node-major cat(xi,xh), t-tiled
            xcT = pk0[:, N : 2 * N]             # feature-major
            identb = pk0[:, 2 * N : 2 * N + 128]
            for p in range(4):
                a0_pair(p)
            for q in range(2):   # a1 in quads
                nc.sync.dma_start(
                    a1sb[:, 4 * q : 4 * q + 4, :],
                    a1t_d[512 * q : 512 * (q + 1), :].rearrange(
                        "(t p) n -> p t n", p=128),
                )
            wpk = sb.tile([128, WPK], BF16, tag="wpk")
            nc.sync.dma_start(wpk[:], wpk_d[:])
            fpk = sb.tile([128, 4], F32, tag="fpk")
            nc.sync.dma_start(fpk[:], fpk_d[:])
            xhT0 = sb.tile([D, N], F32, tag="xhT0")
            nc.sync.dma_start(xhT0[:], xht_d[:])

            def xct(t):
                return xc[:, t * 128 : (t + 1) * 128]

            # ---------------- gconv2 xi-half transposes (idle front PE) ----
            xc2T = sb.tile([128, N], BF16, tag="xc2T")
            nc.vector.tensor_copy(xc2T[0:D, :], xcT[0:D, :])
            nm_x2 = nmp.tile([128, NT, 128], BF16, tag="nm")

            def tr_tiles(nm, srcT, ts, parts=128, foff=0):
                """PE transposes, 4 packed per PSUM bank tile + one DVE evac
                per group (gpsimd cannot read PSUM)."""
                ts = list(ts)
                for i in range(0, len(ts), 4):
                    ptr = pd.tile([128, 4, 128], BF16, tag="pdb")
                    grp = ts[i : i + 4]
                    for k, t in enumerate(grp):
                        nc.tensor.transpose(
                            ptr[:, k, 0:parts],
                            srcT[0:parts, t * 128 : (t + 1) * 128],
                            identb[0:parts, 0:parts])
                    nc.vector.tensor_copy(
                        nm[:, grp[0] : grp[0] + len(grp), foff : foff + parts],
                        ptr[:, 0 : len(grp), 0:parts])

            tr_tiles(nm_x2, xc2T, range(NT), parts=D)

            # ---------------- degree sums (both from a0) --------------------
            #   d0[i] = rowsum(a0t)[i] -> Act accum sums, node-major
            #   d1[i] = colsum(a0t)[i] -> DVE pairwise adds + all-reduce
            dnm = sb.tile([128, NT], F32, tag="dnm")
            dinv_nm = sb.tile([128, 32], F32, tag="dinv_nm")
            dscr0 = sb.tile([128, N], BF16, tag="dscr0")
            dscr1 = sb.tile([128, N], BF16, tag="dscr1")
            nc.vector.memset(dinv_nm[:, NT:32], 1.0)
            Copy = mybir.ActivationFunctionType.Copy
            for t in range(NT):
                nc.scalar.activation(
                    (dscr0 if t % 2 == 0 else dscr1)[:], a0sb[:, t, :], Copy,
                    accum_out=dnm[:, t : t + 1])
            dp01 = sb.tile([128, N], BF16, tag="dp01")
            dp23 = sb.tile([128, N], BF16, tag="dp23")
            ds03 = sb.tile([128, N], BF16, tag="ds03")
            dp45 = sb.tile([128, N], BF16, tag="dp45")
            dp67 = sb.tile([128, N], BF16, tag="dp67")
            ds47 = sb.tile([128, N], BF16, tag="ds47")
            d1s = sb.tile([128, N], BF16, tag="d1s")
            nc.vector.tensor_add(dp01[:], a0sb[:, 0, :], a0sb[:, 1, :])
            nc.vector.tensor_add(dp23[:], a0sb[:, 2, :], a0sb[:, 3, :])
            nc.vector.tensor_add(ds03[:], dp01[:], dp23[:])
            nc.vector.tensor_add(dp45[:], a0sb[:, 4, :], a0sb[:, 5, :])
            nc.vector.tensor_add(dp67[:], a0sb[:, 6, :], a0sb[:, 7, :])
            nc.vector.tensor_add(ds47[:], dp45[:], dp67[:])
            nc.vector.tensor_add(d1s[:], ds03[:], ds47[:])

            # ---------------- hop1 chain0 (streams with a0 DMA) ----------------
            ph1a = ph.tile([128, N], F32, tag="hop")
            for t in range(NT):
                st, sp = (t == 0), (t == NT - 1)
                for c0, c1 in CH:
                    nc.tensor.matmul(ph1a[:, c0:c1], xct(t), a0sb[:, t, c0:c1],
                                     start=st, stop=sp)

            # d0: recip -> 4 DVE 32x32 block transposes -> per-tile broadcasts
            nc.vector.reciprocal(dinv_nm[:, 0:NT], dnm[:])
            dinv0_fm = sb.tile([32, 128], F32, tag="dinv0_fm")
            for b in range(4):
                nc.vector.transpose(
                    dinv0_fm[0:32, b * 32 : (b + 1) * 32],
                    dinv_nm[b * 32 : (b + 1) * 32, 0:32],
                )
            # d1: all-reduce lands the colsum broadcast on every partition
            braw = sb.tile([128, N], F32, tag="braw")
            nc.gpsimd.partition_all_reduce(braw[:], d1s[:], 128,
                                           bass_isa.ReduceOp.add)
            d0row = sb.tile([1, N], F32, tag="d0row")
            nc.gpsimd.dma_start(d0row[0:1, :], dinv0_fm[0:NT, :])
            bc0 = sb.tile([128, N], F32, tag="bc0")
            nc.gpsimd.partition_broadcast(bc0[:], d0row[0:1, :])
            bc1 = sb.tile([128, N], F32, tag="bc1")
            nc.vector.reciprocal(bc1[:], braw[:])

            def zscale(ps, scale):
                """PSUM hop result * per-column 1/d -> SBUF bf16 (chunked)."""
                z = zp.tile([128, N], BF16, tag="z")
                for c0, c1 in CH:
                    nc.vector.tensor_mul(z[:, c0:c1], ps[:, c0:c1],
                                         scale[:, c0:c1])
                return z

            def tr_block(srcT, ts, parts=128):
                """transpose srcT tiles (PE block) -> node-major nm (Pool evac)."""
                nm = nmp.tile([128, NT, 128], BF16, tag="nm")
                tr_tiles(nm, srcT, ts, parts=parts)
                return nm

            def hop_mm(phx, nm, asb, t):
                st, sp = (t == 0), (t == NT - 1)
                for c0, c1 in CH:
                    nc.tensor.matmul(phx[:, c0:c1], nm[:, t, :],
                                     asb[:, t, c0:c1], start=st, stop=sp)

            def proj(pre, w, rhs, stop=False):
                for c0, c1 in CH:
                    nc.tensor.matmul(pre[:, c0:c1], w[:], rhs[:, c0:c1],
                                     start=False, stop=stop)

            # -------- chain1 hop1 (streams with a1 quads), interleaved with --
            # -------- chain0 zscale/transpose/hop2 ---------------------------
            ph1b = ph.tile([128, N], F32, tag="hop")
            z1c0 = zscale(ph1a, bc0)
            nm_a = nmp.tile([128, NT, 128], BF16, tag="nm")
            ph2a = ph.tile([128, N], F32, tag="hop")

            for t in range(4):
                st = (t == 0)
                for c0, c1 in CH:
                    nc.tensor.matmul(ph1b[:, c0:c1], xct(t), a1sb[:, t, c0:c1],
                                     start=st, stop=False)
            tr_tiles(nm_a, z1c0, range(4))
            for t in range(4):
                hop_mm(ph2a, nm_a, a0sb, t)
            for t in range(4, NT):
                sp = (t == NT - 1)
                for c0, c1 in CH:
                    nc.tensor.matmul(ph1b[:, c0:c1], xct(t), a1sb[:, t, c0:c1],
                                     start=False, stop=sp)
            tr_tiles(nm_a, z1c0, range(4, NT))
            for t in range(4, NT):
                hop_mm(ph2a, nm_a, a0sb, t)

            # weights / biases prep (DVE, after wpk+fpk land)
            def wslice(i, o):   # block i of packed weights, width 4*o
                offs = [0, 4 * O1, 8 * O1, 8 * O1 + 4 * O2]
                s = offs[i]
                return [wpk[:, s + m * o : s + (m + 1) * o] for m in range(4)]

            w0m = wslice(0, O1)
            w1m = wslice(1, O1)
            wc0m = wslice(2, O2)
            wc1m = wslice(3, O2)
            wx1 = sb.tile([128, O1], BF16, tag="wx1")
            wz10 = sb.tile([128, O1], BF16, tag="wz10")
            wz20 = sb.tile([128, O1], BF16, tag="wz20")
            wz11 = sb.tile([128, O1], BF16, tag="wz11")
            wz21 = sb.tile([128, O1], BF16, tag="wz21")
            nc.vector.tensor_add(wx1[:], w0m[0], w1m[0])
            nc.vector.tensor_add(wz10[:], w0m[1], w0m[2])
            nc.vector.tensor_copy(wz20[:], w0m[3])
            nc.vector.tensor_add(wz11[:], w1m[1], w1m[2])
            nc.vector.tensor_copy(wz21[:], w1m[3])
            vx1 = sb.tile([128, O2], BF16, tag="vx1")
            vz10 = sb.tile([128, O2], BF16, tag="vz10")
            vz20 = sb.tile([128, O2], BF16, tag="vz20")
            vz11 = sb.tile([128, O2], BF16, tag="vz11")
            vz21 = sb.tile([128, O2], BF16, tag="vz21")
            nc.vector.tensor_add(vx1[:], wc0m[0], wc1m[0])
            nc.vector.tensor_add(vz10[:], wc0m[1], wc0m[2])
            nc.vector.tensor_copy(vz20[:], wc0m[3])
            nc.vector.tensor_add(vz11[:], wc1m[1], wc1m[2])
            nc.vector.tensor_copy(vz21[:], wc1m[3])
            bias1 = sb.tile([O1, 1], F32, tag="bias1")
            bias2 = sb.tile([O2, 1], F32, tag="bias2")
            nc.vector.tensor_add(bias1[:], fpk[:, 0:1], fpk[:, 1:2])
            nc.vector.tensor_add(bias2[:], fpk[0:O2, 2:3], fpk[0:O2, 3:4])

            # chain1 z1 -> transpose -> hop2; projections fill in
            pre1 = pp.tile([O1, N], F32, tag="pre")
            for c0, c1 in CH:
                nc.tensor.matmul(pre1[:, c0:c1], wx1[:], xcT[:, c0:c1],
                                 start=True, stop=False)
            z1c1 = zscale(ph1b, bc1)
            nm_b = tr_block(z1c1, range(NT))
            ph2b = ph.tile([128, N], F32, tag="hop")
            for t in range(NT):
                hop_mm(ph2b, nm_b, a1sb, t)
            z2c0 = zscale(ph2a, bc0)
            proj(pre1, wz10, z1c0)
            proj(pre1, wz11, z1c1)
            proj(pre1, wz20, z2c0)
            z2c1 = zscale(ph2b, bc1)
            proj(pre1, wz21, z2c1, stop=True)

            # sigmoid split into r/u halves; r chunked to unblock gconv2
            Sig = mybir.ActivationFunctionType.Sigmoid
            val_r = sb.tile([D, N], F32, tag="val_r")
            val_u = sb.tile([D, N], F32, tag="val_u")
            rxhT = sb.tile([D, N], BF16, tag="rxhT")
            for c0, c1 in CH:
                nc.scalar.activation(val_r[:, c0:c1], pre1[0:D, c0:c1], Sig,
                                     bias=bias1[0:D, :])
                nc.vector.tensor_mul(rxhT[:, c0:c1], val_r[:, c0:c1],
                                     xhT0[:, c0:c1])

            # ---------------- gconv 2 ----------------
            tr_tiles(nm_x2, rxhT, range(NT), parts=D, foff=D)
            for c0, c1 in CH:
                nc.gpsimd.tensor_copy(xc2T[D:F, c0:c1], rxhT[:, c0:c1])
            nc.scalar.activation(val_u[:], pre1[D:F, :], Sig, bias=bias1[D:F, :])

            ph1a2 = ph.tile([128, N], F32, tag="hop")
            ph1b2 = ph.tile([128, N], F32, tag="hop")
            for t in range(NT):
                hop_mm(ph1a2, nm_x2, a0sb, t)
                hop_mm(ph1b2, nm_x2, a1sb, t)
            pre2 = pp.tile([O2, N], F32, tag="pre")
            for c0, c1 in CH:
                nc.tensor.matmul(pre2[:, c0:c1], vx1[:], xc2T[:, c0:c1],
                                 start=True, stop=False)
            z1c0_2 = zscale(ph1a2, bc0)
            nm_a2 = tr_block(z1c0_2, range(NT))
            ph2a2 = ph.tile([128, N], F32, tag="hop")
            for t in range(NT):
                hop_mm(ph2a2, nm_a2, a0sb, t)
            z1c1_2 = zscale(ph1b2, bc1)
            proj(pre2, vz10, z1c0_2)
            nm_b2 = tr_block(z1c1_2, range(NT))
            ph2b2 = ph.tile([128, N], F32, tag="hop")
            for t in range(NT):
                hop_mm(ph2b2, nm_b2, a1sb, t)
            proj(pre2, vz11, z1c1_2)
            z2c0_2 = zscale(ph2a2, bc0)
            proj(pre2, vz20, z2c0_2)
            z2c1_2 = zscale(ph2b2, bc1)
            proj(pre2, vz21, z2c1_2, stop=True)

            # ---------------- tail: tanh + combine, four pipelined chunks ---
            Tanh = mybir.ActivationFunctionType.Tanh
            cT = sb.tile([O2, N], F32, tag="cT")
            outT = sb.tile([O2, N], F32, tag="outT")
            for i, (c0, c1) in enumerate(CH):
                nc.scalar.activation(cT[:, c0:c1], pre2[:, c0:c1], Tanh,
                                     bias=bias2[:])
                eng = nc.vector if i % 2 == 0 else nc.gpsimd
                eng.tensor_sub(outT[:, c0:c1], xhT0[:, c0:c1], cT[:, c0:c1])
                eng.tensor_mul(outT[:, c0:c1], val_u[:, c0:c1], outT[:, c0:c1])
                eng.tensor_add(outT[:, c0:c1], outT[:, c0:c1], cT[:, c0:c1])
                nc.sync.dma_start(out_d[:, c0:c1], outT[:, c0:c1])

    nc.finalize()
    return nc


def _prep_inputs(inputs, hx, adj, W0, b0, W1, b1, Wc0, bc0, Wc1, bc1):
    """Host-side layout prep -> per-core input maps (no math beyond adj + I)."""
    eye = np.eye(N, dtype=np.float32)
    identb = np.eye(128, dtype=np.float32).astype(NPBF)

    def wblk(W, o):   # [o, 4F] -> [128, 4*o] feat-major blocks
        WT = np.ascontiguousarray(W.T)          # (F*4, o)
        return WT.reshape(128, 4, o).reshape(128, 4 * o)

    wpk = np.concatenate(
        [wblk(W0, O1), wblk(W1, O1), wblk(Wc0, O2), wblk(Wc1, O2)], axis=1
    ).astype(NPBF)
    fpk = np.zeros((128, 4), np.float32)
    fpk[:, 0] = b0
    fpk[:, 1] = b1
    fpk[:O2, 2] = bc0
    fpk[:O2, 3] = bc1
    in_maps = []
    xi_all = inputs.reshape(B, N, D)
    xh_all = hx.reshape(B, N, D)
    for b in range(B):
        adjI = adj[b] + eye
        xcb = np.concatenate([xi_all[b], xh_all[b]], axis=1)  # (N, F)
        xc_nm = xcb.reshape(NT, 128, F).transpose(1, 0, 2).reshape(128, N)
        pk0 = np.concatenate(
            [xc_nm.astype(NPBF), xcb.T.astype(NPBF), identb], axis=1)
        m = {
            "pk0": np.ascontiguousarray(pk0),
            "a1t": np.ascontiguousarray(adjI).astype(NPBF),
            "a0t": np.ascontiguousarray(adjI.T).astype(NPBF),
            "wpk": wpk,
            "fpk": fpk,
            "xht": np.ascontiguousarray(xh_all[b].T).astype(np.float32),
        }
        in_maps.append(m)
    return in_maps


def kernel(**inputs) -> np.ndarray:
    args = {k: np.asarray(v) for k, v in inputs.items()}
    if "nc" not in _cache:
        _cache["nc"] = _build_nc()
    nc = _cache["nc"]
    in_maps = _prep_inputs(
        args["inputs"], args["hx"], args["adj"],
        args["W0"], args["b0"], args["W1"], args["b1"],
        args["Wc0"], args["bc0"], args["Wc1"], args["bc1"],
    )
    res = run_bass_kernel_spmd(nc, in_maps, list(range(B)))
    out = np.stack(
        [np.ascontiguousarray(res.results[b]["out"].T).reshape(N * D)
         for b in range(B)]
    )
    return out.astype(np.float32)
